# revision 13
# baseline (speedup 1.0000x reference)
"""VQ codebook quantizer kernel for 8 Trainium2 NeuronCores.

Math (forward value): out = softmax((2*h@cb.T - ||cb||^2) / TEMP, axis=-1) @ cb
(the ||h||^2 term is constant per row and cancels in softmax; the
straight-through z_q equals z_e in forward value).

Strategy (data-parallel over tokens, codebook replicated):
- 16384 tokens sharded 2048/core across 8 cores.
- Per core, flash-attention-style single pass over K=8192 codes in
  superchunks of 1024, with online max/renormalization:
    mm1 (f32r):  L[t,k] = h.e - ||e||^2/2   (PSUM, tokens on partitions)
    softmax:     DVE chunk max -> running max, ACT exp (scale=20) -> W bf16
                 + fused denominator accumulation
    transpose:   DMA xbar W[t,k] -> WT[k,t] tiles (bf16)
    mm2 (bf16):  Z[t,d] += WT.T @ cb, acc = acc*alpha + Z (DVE)
- f32r (TF32-like, ~1 cyc/row) for mm1; the ||e||^2/2 row is split into
  two rows (10-bit-mantissa high part + residual) appended to the
  contraction so full logit precision survives f32r rounding.
"""

import numpy as np
import ml_dtypes

B, Q, D = 4, 4096, 512
KCODES = 8192
NCORES = 8
T_CORE = (B * Q) // NCORES  # 2048
T_TILES = T_CORE // 128  # 16
KSUP = 1024  # k superchunk
NSUP = KCODES // KSUP  # 8
TEMP = 0.1
SCALE = 2.0 / TEMP  # 20

_CACHE = {}


def _build_nc(reps=1):
    import concourse.bacc as bacc
    import concourse.tile as tile
    from concourse import mybir

    f32 = mybir.dt.float32
    f32r = mybir.dt.float32r
    bf16 = mybir.dt.bfloat16

    nc = bacc.Bacc(None, target_bir_lowering=False)
    hT = nc.dram_tensor("hT", [D, T_CORE], f32r, kind="ExternalInput")
    # rows 0..511 = codebook.T ; row 512 = g_hi ; row 513 = g_lo  (g = -||e||^2/2)
    cbTa = nc.dram_tensor("cbTa", [D + 2, KCODES], f32r, kind="ExternalInput")
    cb16 = nc.dram_tensor("cb16", [KCODES, D], bf16, kind="ExternalInput")
    identd = nc.dram_tensor("identd", [128, 128], bf16, kind="ExternalInput")
    out = nc.dram_tensor("out", [T_CORE, D], f32, kind="ExternalOutput")

    with tile.TileContext(nc) as tc:
        with (
            tc.tile_pool(name="singles", bufs=1) as singles,
            tc.tile_pool(name="cbstream", bufs=2) as cbstream,
            tc.tile_pool(name="wpool", bufs=4) as wpool,
            tc.tile_pool(name="wtpool", bufs=4) as wtpool,
            tc.tile_pool(name="small", bufs=24) as small,
            tc.tile_pool(name="outp", bufs=2) as outp,
            tc.tile_pool(name="psL", bufs=2, space="PSUM") as psL,
            tc.tile_pool(name="psZ", bufs=2, space="PSUM") as psZ,
            tc.tile_pool(name="psT", bufs=2, space="PSUM") as psT,
        ):
            sb_h = singles.tile([128, D // 128, T_CORE], f32r)
            nc.sync.dma_start(sb_h[:], hT.rearrange("(c p) t -> p c t", p=128))
            ident = singles.tile([128, 128], bf16)
            nc.sync.dma_start(ident[:], identd[:])
            sb_ones = singles.tile([2, T_CORE], f32r)
            nc.vector.memset(sb_ones[:].bitcast(f32), 1.0)
            acc = singles.tile([128, T_TILES, D], f32)
            m_run = singles.tile([128, T_TILES], f32)
            denom = singles.tile([128, T_TILES], f32)

            for _ in range(reps):
                nc.vector.memset(acc[:], 0.0)
                nc.vector.memset(m_run[:], -1.0e30)
                nc.vector.memset(denom[:], 0.0)

                # software-pipelined flash loop over (ksup, t_tile)
                n_iter = NSUP * T_TILES
                stage_a = []  # deferred state for stage B

                def do_load(ks):
                    cbt = cbstream.tile(
                        [128, D // 128, KSUP], f32r, name=f"cbt", tag="cbt"
                    )
                    nc.sync.dma_start(
                        cbt[:],
                        cbTa[0:D, ks * KSUP : (ks + 1) * KSUP].rearrange(
                            "(c p) k -> p c k", p=128
                        ),
                    )
                    grow = cbstream.tile([2, KSUP], f32r, name="grow", tag="grow")
                    nc.sync.dma_start(
                        grow[:], cbTa[D : D + 2, ks * KSUP : (ks + 1) * KSUP]
                    )
                    cbv = cbstream.tile(
                        [128, KSUP // 128, D], bf16, name="cbv", tag="cbv"
                    )
                    nc.sync.dma_start(
                        cbv[:],
                        cb16[ks * KSUP : (ks + 1) * KSUP, :].rearrange(
                            "(j p) d -> p j d", p=128
                        ),
                    )
                    return cbt, grow, cbv

                chunks = {}

                def stage_A(i):
                    ks, t = divmod(i, T_TILES)
                    if t == 0:
                        chunks[ks] = do_load(ks)
                        if ks - 2 in chunks:
                            del chunks[ks - 2]
                    cbt, grow, cbv = chunks[ks]
                    tsl = slice(t * 128, (t + 1) * 128)
                    L = psL.tile([128, KSUP], f32, name="L", tag="L")
                    for d in range(D // 128):
                        for j in range(KSUP // 512):
                            nc.tensor.matmul(
                                L[:, j * 512 : (j + 1) * 512],
                                sb_h[:, d, tsl],
                                cbt[:, d, j * 512 : (j + 1) * 512],
                                start=(d == 0),
                                stop=False,
                            )
                    for j in range(KSUP // 512):
                        nc.tensor.matmul(
                            L[:, j * 512 : (j + 1) * 512],
                            sb_ones[:, tsl],
                            grow[:, j * 512 : (j + 1) * 512],
                            start=False,
                            stop=True,
                        )
                    cmax = small.tile([128, 1], f32, name="cmax", tag="cmax")
                    nc.vector.reduce_max(cmax[:], L[:], axis=mybir.AxisListType.X)
                    mo = m_run[:, t : t + 1]
                    dm = small.tile([128, 1], f32, name="dm", tag="dm")
                    nc.vector.tensor_sub(dm[:], mo, cmax[:])  # m_old - cmax
                    nc.vector.tensor_scalar_min(dm[:], dm[:], 0.0)  # m_old - m_new
                    nc.vector.tensor_max(mo, mo, cmax[:])  # m_new (in place)
                    alpha = small.tile([128, 1], f32, name="alpha", tag="alpha")
                    nc.scalar.activation(
                        alpha[:], dm[:], mybir.ActivationFunctionType.Exp, scale=SCALE
                    )
                    nbias = small.tile([128, 1], f32, name="nbias", tag="nbias")
                    nc.vector.tensor_scalar_mul(nbias[:], mo, -SCALE)
                    W = wpool.tile([128, KSUP], bf16, name="W", tag="W")
                    s_chunk = small.tile([128, 1], f32, name="s_chunk", tag="s_chunk")
                    nc.scalar.activation(
                        W[:],
                        L[:],
                        mybir.ActivationFunctionType.Exp,
                        bias=nbias[:],
                        scale=SCALE,
                        accum_out=s_chunk[:],
                    )
                    # denom = denom*alpha + s_chunk
                    nc.vector.scalar_tensor_tensor(
                        denom[:, t : t + 1],
                        denom[:, t : t + 1],
                        alpha[:],
                        s_chunk[:],
                        op0=mybir.AluOpType.mult,
                        op1=mybir.AluOpType.add,
                    )
                    # The DMA xbar transpose intermittently corrupts (reads
                    # early / doubles) on this silicon even from DRAM, so
                    # transpose W on the PE instead (bf16, 1 cyc/row) and
                    # copy PSUM->SBUF on the scalar engine.
                    WT = wtpool.tile([128, KSUP // 128, 128], bf16, name="WT", tag="WT")
                    for j in range(KSUP // 128):
                        pwt = psT.tile([128, 128], bf16, name="pwt", tag="pwt")
                        nc.tensor.transpose(
                            pwt[:], W[:, j * 128 : (j + 1) * 128], ident[:]
                        )
                        nc.scalar.copy(WT[:, j, :], pwt[:])
                    return (t, WT, cbv, alpha)

                def stage_B(state):
                    t, WT, cbv, alpha = state
                    Z = psZ.tile([128, D], f32, name="Z", tag="Z")
                    for j in range(KSUP // 128):
                        nc.tensor.matmul(
                            Z[:],
                            WT[:, j, :],
                            cbv[:, j, :],
                            start=(j == 0),
                            stop=(j == KSUP // 128 - 1),
                        )
                    # acc = acc*alpha + Z
                    nc.vector.scalar_tensor_tensor(
                        acc[:, t, :],
                        acc[:, t, :],
                        alpha[:],
                        Z[:],
                        op0=mybir.AluOpType.mult,
                        op1=mybir.AluOpType.add,
                    )

                SKEW = 2
                pending = []
                for i in range(n_iter):
                    pending.append(stage_A(i))
                    if len(pending) > SKEW:
                        stage_B(pending.pop(0))
                for st in pending:
                    stage_B(st)

                # finalize: out = acc / denom
                for t in range(T_TILES):
                    recip = small.tile([128, 1], f32, name="recip", tag="recip")
                    nc.vector.reciprocal(recip[:], denom[:, t : t + 1])
                    o = outp.tile([128, D], f32, name="o", tag="o")
                    nc.scalar.mul(o[:], acc[:, t, :], recip[:])
                    nc.sync.dma_start(out[t * 128 : (t + 1) * 128, :], o[:])

    nc.compile()
    return nc


class _Runner:
    def __init__(self, nc, n_cores):
        import jax
        from jax.sharding import Mesh, PartitionSpec, NamedSharding

        try:
            from jax.experimental.shard_map import shard_map
        except ImportError:
            from jax.shard_map import shard_map
        from concourse import mybir
        import concourse.bass2jax as b2j

        b2j.install_neuronx_cc_hook()
        self._jax = jax
        self._P = PartitionSpec
        self._NS = NamedSharding
        partition_name = nc.partition_id_tensor.name if nc.partition_id_tensor else None
        in_names, out_names, out_avals, zero_outs = [], [], [], []
        for alloc in nc.m.functions[0].allocations:
            if not isinstance(alloc, mybir.MemoryLocationSet):
                continue
            name = alloc.memorylocations[0].name
            if alloc.kind == "ExternalInput":
                if name != partition_name:
                    in_names.append(name)
            elif alloc.kind == "ExternalOutput":
                shape = tuple(alloc.tensor_shape)
                dtype = mybir.dt.np(alloc.dtype)
                out_names.append(name)
                out_avals.append(jax.core.ShapedArray(shape, dtype))
                zero_outs.append(np.zeros(shape, dtype))
        self.n_params = len(in_names)
        self.n_outs = len(out_avals)
        self.in_names = list(in_names)
        self.out_names = out_names
        self.out_avals = out_avals
        self.zero_outs = zero_outs
        self.n_cores = n_cores
        all_names = in_names + out_names
        if partition_name is not None:
            all_names.append(partition_name)

        def _body(*args):
            operands = list(args)
            if partition_name is not None:
                operands.append(b2j.partition_id_tensor())
            outs = b2j._bass_exec_p.bind(
                *operands,
                out_avals=tuple(out_avals),
                in_names=tuple(all_names),
                out_names=tuple(out_names),
                lowering_input_output_aliases=(),
                sim_require_finite=True,
                sim_require_nnan=True,
                nc=nc,
            )
            return tuple(outs)

        donate = tuple(range(self.n_params, self.n_params + self.n_outs))
        devices = jax.devices()[:n_cores]
        assert len(devices) == n_cores, f"need {n_cores} cores, have {jax.devices()}"
        self.mesh = Mesh(np.asarray(devices), ("core",))
        in_specs = (PartitionSpec("core"),) * (self.n_params + self.n_outs)
        out_specs = (PartitionSpec("core"),) * self.n_outs
        self.sharded = jax.jit(
            shard_map(
                _body,
                mesh=self.mesh,
                in_specs=in_specs,
                out_specs=out_specs,
                check_rep=False,
            ),
            donate_argnums=donate,
            keep_unused=True,
        )

    def concat_inputs(self, in_maps, device_resident=True):
        arrs = [
            np.concatenate(
                [np.asarray(in_maps[c][n]) for c in range(self.n_cores)], axis=0
            )
            for n in self.in_names[: self.n_params]
        ]
        if device_resident:
            sh = self._NS(self.mesh, self._P("core"))
            arrs = [self._jax.device_put(a, sh) for a in arrs]
            self._jax.block_until_ready(arrs)
        return arrs

    def __call__(self, concat_in):
        concat_zeros = [
            np.zeros((self.n_cores * z.shape[0], *z.shape[1:]), z.dtype)
            for z in self.zero_outs
        ]
        out_arrs = self.sharded(*concat_in, *concat_zeros)
        self._jax.block_until_ready(out_arrs)
        return out_arrs

    def split_outputs(self, out_arrs):
        return [
            {
                name: np.asarray(out_arrs[i]).reshape(
                    self.n_cores, *self.out_avals[i].shape
                )[c]
                for i, name in enumerate(self.out_names)
            }
            for c in range(self.n_cores)
        ]


def _mask10(x):
    """Round-to-zero to 10 explicit mantissa bits (safe under f32r's rounding)."""
    u = x.astype(np.float32).view(np.uint32)
    u = u & np.uint32(0xFFFFE000)
    return u.view(np.float32)


def _prep_inputs(h, codebook):
    tokens = np.ascontiguousarray(h.reshape(B * Q, D), dtype=np.float32)
    cb = np.ascontiguousarray(codebook, dtype=np.float32)
    g = (-0.5 * (cb.astype(np.float64) ** 2).sum(axis=1)).astype(np.float32)
    g_hi = _mask10(g)
    g_lo = (g - g_hi).astype(np.float32)
    cbTa = np.concatenate([cb.T, g_hi[None, :], g_lo[None, :]], axis=0)
    cbTa = np.ascontiguousarray(cbTa, dtype=np.float32)
    cb16 = cb.astype(ml_dtypes.bfloat16)
    ident = np.eye(128, dtype=ml_dtypes.bfloat16)
    in_maps = []
    for c in range(NCORES):
        hT_c = np.ascontiguousarray(tokens[c * T_CORE : (c + 1) * T_CORE].T)
        in_maps.append({"hT": hT_c, "cbTa": cbTa, "cb16": cb16, "identd": ident})
    return in_maps


def get_runner(reps=1):
    key = ("runner", reps)
    if key not in _CACHE:
        nc = _build_nc(reps)
        _CACHE[key] = _Runner(nc, NCORES)
    return _CACHE[key]


def kernel(h, codebook):
    h = np.asarray(h)
    codebook = np.asarray(codebook)
    assert h.shape == (B, Q, D) and codebook.shape == (KCODES, D)
    r = get_runner(reps=1)
    in_maps = _prep_inputs(h, codebook)
    outs = r.split_outputs(r(r.concat_inputs(in_maps)))
    full = np.concatenate([outs[c]["out"] for c in range(NCORES)], axis=0)
    return full.reshape(B, Q, D).astype(np.float32)


if __name__ == "__main__":
    rng = np.random.default_rng(0)
    h = rng.standard_normal((B, Q, D), dtype=np.float32)
    cb = rng.standard_normal((KCODES, D), dtype=np.float32)
    z = kernel(h, cb)
    print("out shape", z.shape, "finite:", np.isfinite(z).all())
